# revision 21
# baseline (speedup 1.0000x reference)
"""AttentionLSTMCell Trainium2 kernel.

Key insights:
  1. The attention scores are softmax(h@W1_y.T@w2 + ctx_proj@w2 + const) over S;
     the h-term and const are constant across S, so softmax is INDEPENDENT of the
     LSTM state: attn[t] and weighted[t] are the same for every t. They reduce to
     one tiny precomputation: attn0 = softmax_s(xt[b,s,:]@u), u = W1_x.T@w2.
  2. Only final (h, c) are needed from the recurrence -> one sequential LSTM loop.
  3. x_t @ W_ih.T is hoisted out of the loop as one big fp32r matmul (X_proj).
  4. Recurrent matmul h@W_hh.T runs in bf16 (final error ~1.6e-3, LSTM is
     contractive) in a "fold-4" layout: PSUM partition p = 32*q + b so the
     elementwise LSTM cell runs on 128-partition tiles, produced directly by
     column-group-packed matmuls (tile_position=(0,32q)).

Sharding: the recurrence is replicated on all 8 cores (it is latency-bound, not
throughput-bound); attention/weighted outputs are data-parallel over batch
(4 rows per core); host gathers and broadcasts over T.
"""
import numpy as np
import ml_dtypes

import concourse.bass as bass
import concourse.mybir as mybir
import concourse.tile as tile
from concourse import bacc
from concourse import bass_utils

F32 = mybir.dt.float32
F32R = mybir.dt.float32r
BF16 = mybir.dt.bfloat16
AF = mybir.ActivationFunctionType
ALU = mybir.AluOpType

T, B, S = 32, 32, 64
I = H = C = 1024
A = 128
G4 = 4 * H
NCORES = 8
BL = B // NCORES  # batch rows per core for attention

# gate order in PSUM free dim matches pytorch: i | f | g | o
GATES = [(0, 0), (256, 1024), (512, 2048), (768, 3072)]  # (ps_off, wh_col_base)

_cache = {}
DEBUG_TAPS = False
PHASES = "loop,attn"


def _build():
    nc = bacc.Bacc("TRN2", target_bir_lowering=False, debug=False,
                   num_devices=NCORES)

    XT_d = nc.dram_tensor("XT", [I, T * B], F32R, kind="ExternalInput")
    WI_d = nc.dram_tensor("WI", [I, G4], F32R, kind="ExternalInput")
    WH_d = nc.dram_tensor("WH", [H, G4], BF16, kind="ExternalInput")
    BIH_d = nc.dram_tensor("BIH", [1, G4], F32, kind="ExternalInput")
    BHH_d = nc.dram_tensor("BHH", [1, G4], F32, kind="ExternalInput")
    ONES_d = nc.dram_tensor("ONES", [1, 128], F32R, kind="ExternalInput")
    IDENT_d = nc.dram_tensor("IDENT", [128, 128], F32, kind="ExternalInput")
    W1X_d = nc.dram_tensor("W1X", [A, C], F32, kind="ExternalInput")
    W2C_d = nc.dram_tensor("W2C", [A, 1], F32, kind="ExternalInput")
    ZHT_d = nc.dram_tensor("ZHT", [128, 256], BF16, kind="ExternalInput")
    CTX_d = nc.dram_tensor("CTX", [S, BL, C], F32, kind="ExternalInput")

    HOUT_d = nc.dram_tensor("HOUT", [B, H], F32, kind="ExternalOutput")
    COUT_d = nc.dram_tensor("COUT", [B, H], F32, kind="ExternalOutput")
    WOUT_d = nc.dram_tensor("WOUT", [BL, C], F32, kind="ExternalOutput")
    AOUT_d = nc.dram_tensor("AOUT", [BL, S], F32, kind="ExternalOutput")
    DBG_h = DBG_ht = DBG_ps = None
    if DEBUG_TAPS:
        DBG_h = nc.dram_tensor("DBGH", [2, 128, 256], F32, kind="ExternalOutput")
        DBG_ht = nc.dram_tensor("DBGHT", [128, 256], F32, kind="ExternalOutput")
        DBG_ps = nc.dram_tensor("DBGPS", [128, 1024], F32, kind="ExternalOutput")

    with tile.TileContext(nc) as tc:
        # ---------- persistent pools ----------
        with tc.tile_pool(name="const", bufs=1) as const, \
             tc.tile_pool(name="wh", bufs=1) as whp, \
             tc.tile_pool(name="dram", bufs=1, space="DRAM") as dram:

            # W_hhT resident, bf16: [p, k, col] = W_hhT[128k+p, col]
            WH_sb = whp.tile([128, 8, G4], BF16)
            for k in range(8):
                nc.sync.dma_start(out=WH_sb[:, k, :],
                                  in_=WH_d.ap()[128 * k:128 * (k + 1), :])

            ident_f = const.tile([128, 128], F32)
            nc.sync.dma_start(out=ident_f, in_=IDENT_d.ap())
            ident_bf = const.tile([128, 128], BF16)
            nc.gpsimd.dma_start(out=ident_bf, in_=IDENT_d.ap())  # casting DMA
            ones_sb = const.tile([1, 128], F32R)
            nc.sync.dma_start(out=ones_sb, in_=ONES_d.ap())

            # bias row = b_ih + b_hh, rounded to f32r for the K=1 bias matmul
            bih = const.tile([1, G4], F32)
            bhh = const.tile([1, G4], F32)
            nc.sync.dma_start(out=bih, in_=BIH_d.ap())
            nc.sync.dma_start(out=bhh, in_=BHH_d.ap())
            brow = const.tile([1, G4], F32R)
            nc.vector.tensor_add(brow[:], bih[:], bhh[:])

            # X_proj DRAM staging, raw layout: XF[tb, n] = xp[t, b, n] + bias[n]
            XF = dram.tile([T * B, G4], F32)

            # ---------- u = W1_x.T @ w2 (for attention) ----------
            u_dram = dram.tile([1, C], F32)
            with tc.tile_pool(name="ups", bufs=1, space="PSUM") as ups, \
                 tc.tile_pool(name="usb", bufs=1) as usb:
                w1x_sb = usb.tile([128, C], F32)
                w2c_sb = usb.tile([128, 1], F32)
                nc.sync.dma_start(out=w1x_sb, in_=W1X_d.ap())
                nc.sync.dma_start(out=w2c_sb, in_=W2C_d.ap())
                u_ps = ups.tile([128, 8], F32)
                for j in range(8):
                    nc.tensor.matmul(u_ps[:, j:j + 1],
                                     w1x_sb[:, 128 * j:128 * (j + 1)],
                                     w2c_sb[:], start=True, stop=True)
                u_sb = usb.tile([128, 8], F32)
                nc.vector.tensor_copy(u_sb[:], u_ps[:])
                # u[c] at [c%128, c//128] -> DRAM linear [1, 1024]
                u_out = bass.AP(tensor=u_dram.tensor, offset=u_dram.offset,
                                ap=[[1, 128], [128, 8]])
                nc.sync.dma_start(out=u_out, in_=u_sb[:])

            # ---------- X_proj = inputs @ W_ih.T + bias ----------
            with tc.tile_pool(name="xt", bufs=1) as xtp, \
                 tc.tile_pool(name="wi", bufs=2) as wip, \
                 tc.tile_pool(name="xps", bufs=2, space="PSUM") as xps, \
                 tc.tile_pool(name="xsb", bufs=3) as xsb:
                XT_sb = xtp.tile([128, 8, T * B], F32R)
                for k in range(8):
                    nc.sync.dma_start(out=XT_sb[:, k, :],
                                      in_=XT_d.ap()[128 * k:128 * (k + 1), :])
                for m in range(8):          # 512-wide slice of the 4096 gate cols
                    wi_m = wip.tile([128, 8, 512], F32R, tag="wi")
                    for k in range(8):
                        nc.sync.dma_start(
                            out=wi_m[:, k, :],
                            in_=WI_d.ap()[128 * k:128 * (k + 1),
                                          512 * m:512 * (m + 1)])
                    gate = m // 2
                    q0 = (2 * m) % 4
                    for tb in range(8):
                        ps = xps.tile([128, 512], F32, tag="xp")
                        for k in range(8):
                            nc.tensor.matmul(ps[:],
                                             XT_sb[:, k, 128 * tb:128 * (tb + 1)],
                                             wi_m[:, k, :],
                                             start=(k == 0), stop=False)
                        nc.tensor.matmul(ps[:], ones_sb[:],
                                         brow[:, 512 * m:512 * (m + 1)],
                                         start=False, stop=True)
                        sx = xsb.tile([128, 512], F32, tag="sx")
                        nc.scalar.copy(sx[:], ps[:])
                        nc.sync.dma_start(
                            out=XF[128 * tb:128 * (tb + 1),
                                   512 * m:512 * (m + 1)],
                            in_=sx[:])

            # ---------- LSTM recurrence ----------
            if "loop" in PHASES:
              with tc.tile_pool(name="ht", bufs=2) as htp, \
                 tc.tile_pool(name="cst", bufs=2) as csp, \
                 tc.tile_pool(name="xf", bufs=3) as xfp, \
                 tc.tile_pool(name="ew", bufs=2) as ew, \
                 tc.tile_pool(name="gps", bufs=2, space="PSUM") as gps, \
                 tc.tile_pool(name="tps", bufs=2, space="PSUM") as tps:

                # hT layout [p, j, q, b]: chunk k=2q+j of h.T lives at [:, j, q, :]
                hT = htp.tile([128, 2, 4, 32], BF16, tag="ht")
                nc.sync.dma_start(out=hT[:],
                                  in_=ZHT_d.ap().rearrange("p (j q b) -> p j q b",
                                                           j=2, q=4))
                c_prev = csp.tile([128, 256], F32, tag="c")
                nc.vector.memset(c_prev[:], 0.0)

                h_bf = None
                for t in range(T):
                    xp = xfp.tile([128, 1024], F32, tag="xf")
                    for q in range(4):
                        in_ap = bass.AP(
                            tensor=XF.tensor,
                            offset=XF.offset + (32 * t) * G4 + 256 * q,
                            ap=[[G4, 32], [1024, 4], [1, 256]])
                        nc.sync.dma_start(out=xp[32 * q:32 * (q + 1), :],
                                          in_=in_ap)

                    PS = gps.tile([128, 1024], F32, tag="g")
                    # group order: k-innermost (one accumulation group open at
                    # a time); consecutive groups alternate PSUM bank and
                    # col-group so PE streams overlap via distinct XBUSes.
                    for gi, q in [(0, 0), (2, 1), (1, 2), (3, 3),
                                  (0, 1), (2, 0), (1, 3), (3, 2),
                                  (0, 2), (2, 3), (1, 0), (3, 1),
                                  (0, 3), (2, 2), (1, 1), (3, 0)]:
                        ps_off, wh_base = GATES[gi]
                        for k in range(8):
                            nc.tensor.matmul(
                                PS[32 * q:32 * (q + 1),
                                   ps_off:ps_off + 256],
                                hT[:, k % 2, k // 2, :],
                                WH_sb[:, k, wh_base + 256 * q:
                                      wh_base + 256 * (q + 1)],
                                start=(k == 0), stop=(k == 7),
                                tile_position=(0, 32 * q))

                    g_if = ew.tile([128, 512], F32, tag="gif")
                    nc.vector.tensor_add(g_if[:], PS[:, 0:512], xp[:, 0:512])
                    s_if = ew.tile([128, 512], F32, tag="sif")
                    nc.scalar.activation(s_if[:], g_if[:], AF.Sigmoid)
                    g_og = ew.tile([128, 512], F32, tag="gog")
                    nc.vector.tensor_add(g_og[:], PS[:, 512:1024], xp[:, 512:1024])
                    t_g = ew.tile([128, 256], F32, tag="tg")
                    nc.scalar.activation(t_g[:], g_og[:, 0:256], AF.Tanh)
                    s_o = ew.tile([128, 256], F32, tag="so")
                    nc.scalar.activation(s_o[:], g_og[:, 256:512], AF.Sigmoid)
                    aa = ew.tile([128, 256], F32, tag="aa")
                    nc.vector.tensor_mul(aa[:], s_if[:, 256:512], c_prev[:])
                    bb_ = ew.tile([128, 256], F32, tag="bb")
                    nc.vector.tensor_mul(bb_[:], s_if[:, 0:256], t_g[:])
                    c_new = csp.tile([128, 256], F32, tag="c")
                    nc.vector.tensor_add(c_new[:], aa[:], bb_[:])
                    c_prev = c_new
                    t_c = ew.tile([128, 256], F32, tag="tc")
                    nc.scalar.activation(t_c[:], c_new[:], AF.Tanh)
                    h_bf = ew.tile([128, 256], BF16, tag="hbf")
                    nc.vector.tensor_mul(h_bf[:], s_o[:], t_c[:])
                    if DEBUG_TAPS and t < 2:
                        dt_ = ew.tile([128, 256], F32, tag="dbg")
                        nc.vector.tensor_mul(dt_[:], s_o[:], t_c[:])
                        nc.sync.dma_start(out=DBG_h.ap()[t], in_=dt_[:])
                    if DEBUG_TAPS and t == 1:
                        dps = ew.tile([128, 1024], F32, tag="dps")
                        nc.vector.tensor_copy(dps[:], PS[:])
                        nc.sync.dma_start(out=DBG_ps.ap(), in_=dps[:])

                    if t < T - 1:
                        # full-tile transposes from partition 0 (HW-legal):
                        # PT_j[c', (q,b)] = h_bf[(q,b), 128j + c'] -> hT[:, j, q, b]
                        hT = htp.tile([128, 2, 4, 32], BF16, tag="ht")
                        for j in range(2):
                            PT = tps.tile([128, 128], BF16, tag="pt")
                            nc.tensor.transpose(
                                PT[:], h_bf[:, 128 * j:128 * (j + 1)],
                                ident_bf[:])
                            nc.vector.tensor_copy(
                                hT[:, j, :, :],
                                PT[:].rearrange("p (q b) -> p q b", q=4))
                        if DEBUG_TAPS and t == 0:
                            dh = ew.tile([128, 256], F32, tag="dbg")
                            nc.vector.tensor_copy(dh[:], PT[:])
                            nc.sync.dma_start(out=DBG_ht.ap(), in_=dh[:])
                    else:
                        # final step: also produce fp32 h for output
                        h_f = ew.tile([128, 256], F32, tag="hf")
                        nc.vector.tensor_mul(h_f[:], s_o[:], t_c[:])
                        for q in range(4):
                            hout_ap = bass.AP(
                                tensor=HOUT_d, offset=256 * q,
                                ap=[[1024, 32], [1, 256]])
                            nc.sync.dma_start(out=hout_ap,
                                              in_=h_f[32 * q:32 * (q + 1), :])
                            cout_ap = bass.AP(
                                tensor=COUT_d, offset=256 * q,
                                ap=[[1024, 32], [1, 256]])
                            nc.sync.dma_start(out=cout_ap,
                                              in_=c_new[32 * q:32 * (q + 1), :])

            # ---------- attention (state-independent), 4 batch rows ----------
            if "attn" in PHASES:
              with tc.tile_pool(name="at", bufs=1) as at, \
                 tc.tile_pool(name="aps", bufs=2, space="PSUM") as aps:
                xt_sb = at.tile([S, BL, C], F32)
                nc.sync.dma_start(out=xt_sb[:], in_=CTX_d.ap())
                ub_sb = at.tile([S, C], F32)
                if "nobc" in PHASES:
                    nc.vector.tensor_copy(ub_sb[:], xt_sb[:, 0, :].bitcast(F32))
                else:
                    nc.sync.dma_start(out=ub_sb[:],
                                      in_=u_dram[0, :].partition_broadcast(S))
                cs_cols = at.tile([S, BL], F32)
                for bb in range(BL):
                    mm_out = at.tile([S, C], F32, tag="mmout")
                    nc.vector.tensor_mul(mm_out[:], xt_sb[:, bb, :],
                                         ub_sb[:])
                    nc.vector.reduce_sum(cs_cols[:, bb:bb + 1], mm_out[:],
                                         axis=mybir.AxisListType.X)
                if "a1" in PHASES:
                    a1out = bass.AP(tensor=AOUT_d, offset=0,
                                    ap=[[1, S], [S, BL]])
                    nc.sync.dma_start(out=a1out, in_=cs_cols[:])
                if "a1" not in PHASES:
                  cs_dram = dram.tile([S, BL], F32)
                  nc.sync.dma_start(out=cs_dram[:], in_=cs_cols[:])
                  cs_t = at.tile([BL, S], F32)
                  csT_ap = bass.AP(tensor=cs_dram.tensor, offset=cs_dram.offset,
                                   ap=[[1, BL], [BL, S]])
                  nc.sync.dma_start(out=cs_t[:], in_=csT_ap)
                  negmax = at.tile([BL, 1], F32)
                  nc.vector.reduce_max(negmax[:], cs_t[:],
                                       axis=mybir.AxisListType.X, negate=True)
                  e_t = at.tile([BL, S], F32)
                  nc.scalar.activation(e_t[:], cs_t[:], AF.Exp,
                                       bias=negmax[:, 0:1])
                  ssum = at.tile([BL, 1], F32)
                  nc.vector.reduce_sum(ssum[:], e_t[:], axis=mybir.AxisListType.X)
                  rinv = at.tile([BL, 1], F32)
                  nc.vector.reciprocal(rinv[:], ssum[:])
                  attn_f = at.tile([BL, S], F32)
                  nc.vector.tensor_scalar_mul(attn_f[:], e_t[:], rinv[:, 0:1])
                  nc.sync.dma_start(out=AOUT_d.ap(), in_=attn_f[:])
                  at_dram = dram.tile([BL, S], F32)
                  nc.sync.dma_start(out=at_dram[:], in_=attn_f[:])
                  attnT = at.tile([S, BL], F32)
                  aT_ap = bass.AP(tensor=at_dram.tensor, offset=at_dram.offset,
                                  ap=[[1, S], [S, BL]])
                  nc.sync.dma_start(out=attnT[:], in_=aT_ap)
                  for bb in range(BL):
                    wps = aps.tile([1, C], F32, tag="wps")
                    for half in range(2):
                        nc.tensor.matmul(
                            wps[:, 512 * half:512 * (half + 1)],
                            attnT[:, bb:bb + 1],
                            xt_sb[:, bb, 512 * half:512 * (half + 1)],
                            start=True, stop=True)
                    w_sb = at.tile([1, C], F32, tag="wsb")
                    nc.vector.tensor_copy(w_sb[:], wps[:])
                    nc.sync.dma_start(out=WOUT_d.ap()[bb:bb + 1, :], in_=w_sb[:])

    nc.compile()
    return nc


def kernel(inputs, context, W_ih, W_hh, b_ih, b_hh, W1, b1, W2, b2):
    if "nc" not in _cache:
        _cache["nc"] = _build()
    nc = _cache["nc"]

    f32 = np.float32
    XT = np.ascontiguousarray(inputs.reshape(T * B, I).T).astype(f32, copy=False)
    WI = np.ascontiguousarray(W_ih.T).astype(f32, copy=False)
    WH = np.ascontiguousarray(W_hh.T).astype(ml_dtypes.bfloat16)
    base = {
        "XT": XT,
        "WI": WI,
        "WH": WH,
        "BIH": np.ascontiguousarray(b_ih[None, :]).astype(f32, copy=False),
        "BHH": np.ascontiguousarray(b_hh[None, :]).astype(f32, copy=False),
        "ONES": np.ones((1, 128), f32),
        "IDENT": np.eye(128, dtype=f32),
        "W1X": np.ascontiguousarray(W1[:, H:]).astype(f32, copy=False),
        "W2C": np.ascontiguousarray(W2[0][:, None]).astype(f32, copy=False),
        "ZHT": np.zeros((128, 256), ml_dtypes.bfloat16),
    }
    in_maps = []
    for k in range(NCORES):
        m = dict(base)
        m["CTX"] = np.ascontiguousarray(
            context[:, BL * k:BL * (k + 1), :]).astype(f32, copy=False)
        in_maps.append(m)

    import os
    trace = os.environ.get("KTRACE", "") == "1"
    res = bass_utils.run_bass_kernel_spmd(nc, in_maps,
                                          core_ids=list(range(NCORES)),
                                          trace=trace)
    rs = res.results
    _cache["last_results"] = res

    h = rs[0]["HOUT"]
    c = rs[0]["COUT"]
    w0 = np.concatenate([rs[k]["WOUT"] for k in range(NCORES)], axis=0)
    a0 = np.concatenate([rs[k]["AOUT"] for k in range(NCORES)], axis=0)
    outputs = np.ascontiguousarray(np.broadcast_to(w0[None], (T, B, C)))
    attentions = np.ascontiguousarray(np.broadcast_to(a0[None], (T, B, S)))
    return outputs, h, c, attentions
